# revision 1
# baseline (speedup 1.0000x reference)
"""DirectedGraphConvolution Trainium2 kernel.

Per batch element b (one per NeuronCore, 8 total, data-parallel):
    N_e = H @ W                          [n, dout]
    T1  = G  @ N_e                       [n, dout]
    T2  = G.T @ N_e                      [n, dout]
    rs  = G.sum(-1); cs = G.sum(-2)
    out = [ relu(0.5*(T1 + T2)),             # G_F @ N_e,  G_F = (G+G.T)/2
            relu(G.T @ (T1 / rs[:,None])),   # G_Sin @ N_e
            relu(G  @ (T2 / cs[:,None])) ]   # G_So  @ N_e
(The reference's [n,n] Gram matrices are never materialized - associativity.)

Layouts: matmul computes out[m,n] = sum_p lhsT[p,m]*rhs[p,n].  G is kept
SBUF-resident in natural layout (row index on partitions), which directly
serves the G.T-left products (passes A and C).  G-left products (pass B)
get their stationary GT blocks from on-the-fly PE transposes, software-
pipelined twelve steps ahead of the consuming matmuls (batching the
transpose_mode instructions cuts PE mode-transition overhead).  cs falls
out of a ones-column fused into pass A's moving operand; rs is reduced
on DVE while G streams in.  Pass A: sweep 1 (jt 0-7) is it-outer across
8 PSUM banks so its matmuls track the G DMA arrival; sweep 2 (jt 8-15)
is jt-outer (G resident by then), which accumulates bank-local and
releases banks progressively so pass B's pipeline starts early.  All
matmuls run in float32r (fp32 storage, ~1 cyc/row at even N>=256).
"""

import numpy as np
import concourse.bass as bass
import concourse.mybir as mybir
import concourse.tile as tile
from concourse import bacc
from concourse.bass_utils import run_bass_kernel_spmd
from concourse.masks import make_identity

F32 = mybir.dt.float32
F32R = mybir.dt.float32r
RELU = mybir.ActivationFunctionType.Relu
AX = mybir.AxisListType.X

P = 128
B = 8
N = 2048
NO = N // P            # 16 row tiles
DIN = 256
DOUT = 256
KO = DIN // P          # 2 k tiles for H @ W
W3 = 3 * DOUT
RB = 2 + DOUT + DOUT   # rhs_b columns: [ones ones | N_e | T2'] (f32r needs even widths)


def build():
    nc = bacc.Bacc("TRN2", target_bir_lowering=False)
    G = nc.declare_dram_parameter("G", [N, N], F32, isOutput=False)
    H = nc.declare_dram_parameter("H", [N, DIN], F32, isOutput=False)
    W = nc.declare_dram_parameter("W", [DIN, DOUT], F32, isOutput=False)
    out = nc.declare_dram_parameter("out", [N, W3], F32, isOutput=True)

    G_r = G.rearrange("(o p) j -> p o j", p=P).bitcast(F32R)
    H_r = H.rearrange("(o p) d -> p o d", p=P).bitcast(F32R)
    W_r = W.rearrange("(o p) d -> p o d", p=P).bitcast(F32R)
    out_r = out.rearrange("(o p) d -> p o d", p=P)

    with tile.TileContext(nc) as tc:
        with (
            tc.tile_pool(name="const", bufs=1) as const,
            tc.tile_pool(name="gpool", bufs=1) as gpool,
            tc.tile_pool(name="big", bufs=1) as big,
            tc.tile_pool(name="hin", bufs=3) as hin,
            tc.tile_pool(name="stage", bufs=4) as stage,
            tc.tile_pool(name="gtp", bufs=12) as gtp,
            tc.tile_pool(name="tmpp", bufs=2) as tmpp,
        ):
            # G DMAs own the Sync HWDGE queue exclusively; everything else
            # (W, H, outputs) issues elsewhere so a slot-release wait can
            # never block the G stream behind it.
            g_tiles = [
                gpool.tile([P, N], F32R, tag=f"g{o}", name=f"g{o}")
                for o in range(NO)
            ]
            for o in range(NO):
                nc.sync.dma_start(g_tiles[o][:, 0:N // 2], G_r[:, o, 0:N // 2])
                nc.sync.dma_start(g_tiles[o][:, N // 2:N], G_r[:, o, N // 2:N])

            w_sb = const.tile([P, KO, DOUT], F32R)
            nc.scalar.dma_start(w_sb, W_r)

            ident_f32 = const.tile([P, P], F32)
            make_identity(nc, ident_f32)
            ident = const.tile([P, P], F32R)
            nc.vector.tensor_copy(ident, ident_f32)
            # rhsb[o] columns: [N_e | T2']
            rhsb = [
                big.tile([P, RB], F32R, tag=f"rb{o}", name=f"rb{o}") for o in range(NO)
            ]
            t1 = [
                big.tile([P, DOUT], F32R, tag=f"t1{o}", name=f"t1{o}")
                for o in range(NO)
            ]
            rsinv = const.tile([P, NO, 1], F32)
            ones_f32 = const.tile([P, 1], F32)
            nc.vector.memset(ones_f32, 1.0)
            cs_sb = const.tile([P, NO, 1], F32)
            csinv = const.tile([P, NO, 1], F32)
            for o in range(NO):
                nc.vector.tensor_copy(rhsb[o][:, 0:1], ones_f32)
                nc.vector.tensor_copy(rhsb[o][:, 1:2], ones_f32)

            # ---- N_e = H @ W  (transpose H blocks on PE, then matmul) ----
            with (
                tc.tile_pool(name="ps_ht", bufs=3, space="PSUM") as ps_ht,
                tc.tile_pool(name="ps_ne", bufs=2, space="PSUM") as ps_ne,
            ):
                # H tiles park in rhsb's T2' region (unused until pass A's
                # epilogue) so every H DMA issues immediately with no SBUF
                # slot-release wait -- a waiting DMA would block the shared
                # HWDGE semaphore slots the G stream cycles through.
                # software pipeline: transposes for tile t run while tile
                # t-1's matmuls consume the previous transposed block, so the
                # PE never stalls on the PSUM->SBUF copy between them
                for t in range(NO):
                    nc.scalar.dma_start(rhsb[t][:, 2 + DOUT:RB], H_r[:, t, :])
                hts = {}
                for t in range(NO + 1):
                    if t < NO:
                        h_t = rhsb[t][:, 2 + DOUT:RB]
                        ht_t = hin.tile([P, KO, P], F32R, tag="ht")
                        for kt in range(KO):
                            pt = ps_ht.tile([P, P], F32, tag="pht")
                            nc.tensor.transpose(
                                pt.bitcast(F32R), h_t[:, kt * P:(kt + 1) * P], ident
                            )
                            nc.vector.tensor_copy(ht_t[:, kt, :], pt.bitcast(F32R))
                        hts[t] = ht_t
                    if t >= 1:
                        u = t - 1
                        ht_u = hts.pop(u)
                        pne = ps_ne.tile([P, DOUT], F32, tag="pne")
                        for kt in range(KO):
                            nc.tensor.matmul(
                                pne,
                                ht_u[:, kt, :],
                                w_sb[:, kt, :],
                                start=(kt == 0),
                                stop=(kt == KO - 1),
                            )
                        nc.vector.tensor_copy(rhsb[u][:, 2:2 + DOUT], pne)

                # rs = row sums (DVE) as G tiles land
                for o in range(NO):
                    rs_t = tmpp.tile([P, 1], F32, tag="rs")
                    nc.vector.reduce_sum(rs_t, g_tiles[o].bitcast(F32), axis=AX)
                    nc.vector.reciprocal(rsinv[:, o, :], rs_t)

            # ---- pass A: [cs cs | T2] = G.T @ [ones ones | N_e] ----
            with tc.tile_pool(name="psA", bufs=8, space="PSUM") as psA:
                def a_epilogue(jt, pa):
                    nc.vector.tensor_copy(cs_sb[:, jt, :], pa[:, 0:1])
                    nc.vector.reciprocal(csinv[:, jt, :], pa[:, 0:1])
                    # T2' = T2 / cs  -> rhsb cols [2+DOUT : RB]
                    nc.vector.tensor_scalar_mul(
                        rhsb[jt][:, 2 + DOUT:RB],
                        pa[:, 2:2 + DOUT],
                        csinv[:, jt, 0:1],
                    )

                # sweep 1 (jt 0-7): it-outer across 8 banks, tracks G arrival
                pas = {
                    jt: psA.tile([P, 2 + DOUT], F32, tag="pa", name=f"pa{jt}")
                    for jt in range(8)
                }
                for it in range(NO):
                    for jt in range(8):
                        nc.tensor.matmul(
                            pas[jt],
                            g_tiles[it][:, jt * P:(jt + 1) * P],
                            rhsb[it][:, 0:2 + DOUT],
                            start=(it == 0),
                            stop=(it == NO - 1),
                        )
                for jt in range(8):
                    a_epilogue(jt, pas[jt])

                # sweep 2 (jt 8-15): G is resident by now, so go jt-outer --
                # consecutive matmuls accumulate into one bank (no per-matmul
                # bank cycling) and banks release progressively, letting pass
                # B's transpose pipeline claim PSUM early
                for jt in range(8, NO):
                    pa2 = psA.tile([P, 2 + DOUT], F32, tag="pa", name=f"pa{jt}")
                    for it in range(NO):
                        nc.tensor.matmul(
                            pa2,
                            g_tiles[it][:, jt * P:(jt + 1) * P],
                            rhsb[it][:, 0:2 + DOUT],
                            start=(it == 0),
                            stop=(it == NO - 1),
                        )
                    a_epilogue(jt, pa2)

            # ---- pass B: [T1 | out3raw] = G @ [N_e | T2'] ----
            # stationary GT blocks from PE transposes, pipelined ahead;
            # PSUM->SBUF block copies alternate DVE / ACT
            with (
                tc.tile_pool(name="psB", bufs=3, space="PSUM") as psB,
                tc.tile_pool(name="psT", bufs=5, space="PSUM") as psT,
            ):
                for it in range(NO):
                    pb = psB.tile([P, 2 * DOUT], F32, tag="pb")
                    gts = {}
                    LOOKAHEAD = 12
                    for step in range(NO + LOOKAHEAD):
                        if step < NO:
                            jt = step
                            pt = psT.tile([P, P], F32, tag="ptr")
                            nc.tensor.transpose(
                                pt.bitcast(F32R),
                                g_tiles[it][:, jt * P:(jt + 1) * P],
                                ident,
                            )
                            gt_t = gtp.tile([P, P], F32R, tag="gt")
                            if jt % 2 == 0:
                                nc.vector.tensor_copy(gt_t, pt.bitcast(F32R))
                            else:
                                nc.scalar.copy(gt_t, pt.bitcast(F32R))
                            gts[jt] = gt_t
                        if step >= LOOKAHEAD:
                            jt = step - LOOKAHEAD
                            nc.tensor.matmul(
                                pb,
                                gts.pop(jt),
                                rhsb[jt][:, 2:RB],
                                start=(jt == 0),
                                stop=(jt == NO - 1),
                            )
                    # out1 = relu(0.5*(T1 + cs*T2'))
                    t2r = tmpp.tile([P, DOUT], F32, tag="t2r")
                    nc.vector.tensor_scalar_mul(
                        t2r, rhsb[it][:, 2 + DOUT:RB].bitcast(F32), cs_sb[:, it, 0:1]
                    )
                    nc.vector.tensor_add(t2r, t2r, pb[:, 0:DOUT])
                    o1 = stage.tile([P, DOUT], F32, tag="o1")
                    nc.scalar.activation(o1, t2r, RELU, scale=0.5)
                    nc.sync.dma_start(out_r[:, it, 0:DOUT], o1)
                    # T1' = T1 / rs
                    nc.vector.tensor_scalar_mul(
                        t1[it], pb[:, 0:DOUT], rsinv[:, it, 0:1]
                    )
                    # out3 = relu(G @ T2')
                    o3 = stage.tile([P, DOUT], F32, tag="o3")
                    nc.scalar.activation(o3, pb[:, DOUT:2 * DOUT], RELU)
                    nc.sync.dma_start(out_r[:, it, 2 * DOUT:W3], o3)

            # ---- pass C: out2 = relu(G.T @ T1') ----
            with tc.tile_pool(name="psC", bufs=6, space="PSUM") as psC:
                for jt in range(NO):
                    pc = psC.tile([P, DOUT], F32, tag="pc")
                    for it in range(NO):
                        nc.tensor.matmul(
                            pc,
                            g_tiles[it][:, jt * P:(jt + 1) * P],
                            t1[it],
                            start=(it == 0),
                            stop=(it == NO - 1),
                        )
                    o2 = stage.tile([P, DOUT], F32, tag="o2")
                    nc.scalar.activation(o2, pc, RELU)
                    nc.sync.dma_start(out_r[:, jt, DOUT:2 * DOUT], o2)

    nc.compile()
    return nc


_NC = None


def _get_nc():
    global _NC
    if _NC is None:
        _NC = build()
    return _NC


def run(inputs: dict, trace: bool = False):
    """Run on 8 cores; returns (stacked_out [B,N,W3], BassKernelResults)."""
    H, G, W = inputs["H"], inputs["G"], inputs["W"]
    H = np.ascontiguousarray(H, dtype=np.float32)
    G = np.ascontiguousarray(G, dtype=np.float32)
    W = np.ascontiguousarray(W, dtype=np.float32)
    in_maps = [
        {"G": np.ascontiguousarray(G[b]), "H": np.ascontiguousarray(H[b]), "W": W}
        for b in range(B)
    ]
    nc = _get_nc()
    res = run_bass_kernel_spmd(nc, in_maps, core_ids=list(range(B)), trace=trace)
    out = np.stack([res.results[b]["out"] for b in range(B)], axis=0)
    return out, res


def kernel(H, G, W):
    out, _ = run({"H": H, "G": G, "W": W})
    return out



# revision 9
# speedup vs baseline: 1.1078x; 1.1078x over previous
"""DirectedGraphConvolution Trainium2 kernel (bf16 streaming rewrite).

Per batch element b (one per NeuronCore, 8 total, data-parallel):
    N_e = H @ W                          [n, dout]
    T1  = G  @ N_e                       [n, dout]
    T2  = G.T @ N_e   (+ cs via a fused ones-column)
    out = [ relu(0.5*(T1 + T2)),             # G_F @ N_e
            relu(G.T @ (T1 / rs[:,None])),   # G_Sin @ N_e
            relu(G  @ (T2 / cs[:,None])) ]   # G_So  @ N_e

All matmuls run in bf16 (1 cyc/row on PE for matmul AND transpose; fp32
psum accumulate).  G is converted f32->bf16 on arrival by the ACT engine
(which also row-sums it via accum_out -> rs for free); both G (natural)
and G^T (PE-transposed) then stay SBUF-resident in bf16 (8 MB each), so
the G-left products (T1, out3) never re-transpose.  cs falls out of a
ones column fused into pass A's moving operand.

Schedule: the PE is kept continuously busy (TRN2 p-state ramp: any idle
gap drops the PE to 1.2 GHz for 3 us).  While G streams in (~55 us of
HBM time), the PE runs H@W, all 256 G-block transposes, 4 of the 16
pass-A chains, and all 16 T1 row chains; after the stream it runs the 12
remaining pass-A chains, pass C (G^T @ T1'), and pass B3 (G @ T2')
back-to-back with no dependency stalls.  H lands first on the sync DMA
queue; G tiles 0-1 sneak in on the scalar queue so PE work starts ~4 us
in.  Outputs leave on the scalar queue as soon as each 128-row tile is
done.
"""

import numpy as np
import concourse.bass as bass
import concourse.mybir as mybir
import concourse.tile as tile
from concourse import bacc
from concourse.bass_utils import run_bass_kernel_spmd
from concourse.masks import make_identity

F32 = mybir.dt.float32
BF16 = mybir.dt.bfloat16
RELU = mybir.ActivationFunctionType.Relu
COPY = mybir.ActivationFunctionType.Copy

P = 128
B = 8
N = 2048
NO = N // P            # 16 row tiles
DIN = 256
DOUT = 256
KO = DIN // P          # 2 k tiles for H @ W
W3 = 3 * DOUT
RW = 2 * DOUT + 2      # rhs cols: [N_e(256) | one | pad | T2'(256)]
ONE_C = 256            # ones column (for cs via pass A)
T2_C = 258             # T2' start column


def build():
    nc = bacc.Bacc("TRN2", target_bir_lowering=False)
    G = nc.declare_dram_parameter("G", [N, N], F32, isOutput=False)
    H = nc.declare_dram_parameter("H", [N, DIN], F32, isOutput=False)
    W = nc.declare_dram_parameter("W", [DIN, DOUT], F32, isOutput=False)
    out = nc.declare_dram_parameter("out", [N, W3], F32, isOutput=True)

    G_r = G.rearrange("(o p) j -> p o j", p=P)
    H_r = H.rearrange("(o p) d -> p o d", p=P)
    W_r = W.rearrange("(o p) d -> p o d", p=P)
    out_r = out.rearrange("(o p) d -> p o d", p=P)

    with tile.TileContext(nc) as tc:
        with (
            tc.tile_pool(name="const", bufs=1) as const,
            tc.tile_pool(name="gland", bufs=3) as gland,
            tc.tile_pool(name="gbf", bufs=1) as gbfp,
            tc.tile_pool(name="gtp", bufs=1) as gtpp,
            tc.tile_pool(name="rhs", bufs=1) as rhsp,
            tc.tile_pool(name="t1pp", bufs=1) as t1pp,
            tc.tile_pool(name="hland", bufs=1) as hlp,
            tc.tile_pool(name="hbf", bufs=4) as hbfp,
            tc.tile_pool(name="htmp", bufs=4) as htp,
            tc.tile_pool(name="stage", bufs=5) as stage,
            tc.tile_pool(name="tmp", bufs=3) as tmpp,
            tc.tile_pool(name="psA", bufs=1, space="PSUM") as psA,
        ):
            # ---- constants / persistent tiles ----
            ident_f32 = const.tile([P, P], F32)
            make_identity(nc, ident_f32)
            ident = const.tile([P, P], BF16)
            nc.vector.tensor_copy(ident, ident_f32)
            w_bf = const.tile([P, KO, DOUT], BF16)
            rs = const.tile([P, NO], F32)
            rsinv = const.tile([P, NO], F32)
            csinv = const.tile([P, NO], F32)

            g_bf = [gbfp.tile([P, N], BF16, tag=f"g{i}", name=f"g{i}")
                    for i in range(NO)]
            gt = [gtpp.tile([P, NO, P], BF16, tag=f"t{i}", name=f"t{i}")
                  for i in range(NO)]
            rhs = [rhsp.tile([P, RW], BF16, tag=f"r{i}", name=f"r{i}")
                   for i in range(NO)]
            t1p = [t1pp.tile([P, DOUT], BF16, tag=f"p{i}", name=f"p{i}")
                   for i in range(NO)]
            hland = [hlp.tile([P, DIN], F32, tag=f"h{t}", name=f"h{t}")
                     for t in range(NO)]

            # ---- input DMAs ----
            # H first on the sync queue (FIFO: fully lands ~7us so the
            # N_e pipeline never starves); G tiles 0-1 + W on the scalar
            # queue in parallel so PE transposes start ~4us in; G 2..15
            # behind H on sync.  gland ring (3 bufs) throttles the G
            # stream to the ACT converts via slot-release deps.
            w_land = [tmpp.tile([P, DOUT], F32, tag="tm", name=f"wl{k}")
                      for k in range(KO)]
            for k in range(KO):
                nc.scalar.dma_start(w_land[k], W_r[:, k, :])
            for t in range(NO):
                nc.sync.dma_start(hland[t], H_r[:, t, :])
            g_land = []
            for i in range(NO):
                gl = gland.tile([P, N], F32, tag="gl", name=f"gl{i}")
                g_land.append(gl)
                eng = nc.scalar if i < 2 else nc.sync
                eng.dma_start(gl, G_r[:, i, :])

            for k in range(KO):
                nc.vector.tensor_copy(w_bf[:, k, :], w_land[k])
            for i in range(NO):
                nc.vector.memset(rhs[i][:, ONE_C:ONE_C + 1], 1.0)

            with (
                tc.tile_pool(name="psTR", bufs=2, space="PSUM") as psTR,
                tc.tile_pool(name="psB1", bufs=1, space="PSUM") as psB1,
            ):
                pa = [psA.tile([P, DOUT + 1], F32, tag=f"pa{j}", name=f"pa{j}")
                      for j in range(4)]
                psb1 = psB1.tile([P, 2, DOUT], F32)

                # B1 rows per stream block: rows need all of N_e (~block 3)
                # and gt[r] (block r); 2/block until caught up.
                b1_due = {3: [0, 1], 4: [2, 3], 5: [4, 5]}
                for i in range(6, NO):
                    b1_due[i] = [i]

                def emit_hquad(q):
                    # 4 H tiles: transpose blocks, N_e = H @ W
                    tq = psTR.tile([P, 8, P], BF16, tag="tr", name=f"htr{q}")
                    hts = []
                    for j in range(4):
                        t = 4 * q + j
                        hb = hbfp.tile([P, DIN], BF16, tag="hb", name=f"hb{t}")
                        nc.scalar.activation(hb, hland[t], COPY)
                        for kt in range(KO):
                            nc.tensor.transpose(
                                tq[:, 2 * j + kt, :],
                                hb[:, kt * P:(kt + 1) * P], ident)
                        ht = htp.tile([P, KO, P], BF16, tag="ht", name=f"ht{t}")
                        nc.vector.tensor_copy(ht, tq[:, 2 * j:2 * j + 2, :])
                        hts.append(ht)
                    for j in range(4):
                        t = 4 * q + j
                        pne = psb1[:, t % 2, :]
                        for kt in range(KO):
                            nc.tensor.matmul(
                                pne, hts[j][:, kt, :], w_bf[:, kt, :],
                                start=(kt == 0), stop=(kt == KO - 1))
                        nc.vector.tensor_copy(rhs[t][:, 0:DOUT], pne)

                def emit_b1_row(r):
                    pb = psb1[:, r % 2, :]
                    for k in range(NO):
                        nc.tensor.matmul(
                            pb, gt[r][:, k, :], rhs[k][:, 0:DOUT],
                            start=(k == 0), stop=(k == NO - 1))
                    # T1' = T1 / rs  (bf16, pass C moving operand)
                    nc.vector.tensor_scalar_mul(t1p[r], pb, rsinv[:, r:r + 1])

                def emit_a_chain_step(ia):
                    for jt in range(4):
                        nc.tensor.matmul(
                            pa[jt][:, 0:DOUT + 1],
                            g_bf[ia][:, jt * P:(jt + 1) * P],
                            rhs[ia][:, 0:DOUT + 1],
                            start=(ia == 0), stop=(ia == NO - 1))

                # ---- stream loop ----
                for i in range(NO):
                    if i < 4:
                        emit_hquad(i)
                    # f32 -> bf16 convert; rs row-sums fall out of accum_out
                    nc.scalar.activation(g_bf[i], g_land[i], COPY,
                                         accum_out=rs[:, i:i + 1])
                    nc.vector.reciprocal(rsinv[:, i:i + 1], rs[:, i:i + 1])
                    # 16 PE transposes -> G^T blocks (psum bf16), copied out
                    # in 4-block chunks split DVE/Pool
                    ta = psTR.tile([P, 8, P], BF16, tag="tr", name=f"ta{i}")
                    for k in range(8):
                        nc.tensor.transpose(
                            ta[:, k, :], g_bf[i][:, k * P:(k + 1) * P], ident)
                        if k == 3:
                            nc.vector.tensor_copy(gt[i][:, 0:4, :], ta[:, 0:4, :])
                    nc.vector.tensor_copy(gt[i][:, 4:8, :], ta[:, 4:8, :])
                    # pass A chains jt 0-3, two tiles behind arrival so the
                    # N_e pipeline is never a gate
                    if i >= 2:
                        emit_a_chain_step(i - 2)
                    tb = psTR.tile([P, 8, P], BF16, tag="tr", name=f"tb{i}")
                    for k in range(8):
                        nc.tensor.transpose(
                            tb[:, k, :], g_bf[i][:, (8 + k) * P:(9 + k) * P],
                            ident)
                        if k == 3:
                            nc.vector.tensor_copy(gt[i][:, 8:12, :], tb[:, 0:4, :])
                    nc.vector.tensor_copy(gt[i][:, 12:16, :], tb[:, 4:8, :])
                    for r in b1_due.get(i, []):
                        emit_b1_row(r)

                # A chain catch-up (sources 14, 15)
                emit_a_chain_step(NO - 2)
                emit_a_chain_step(NO - 1)

            # ---- post phase: A jt4-15 + epilogues, then C, then B3 ----
            with (
                tc.tile_pool(name="psC", bufs=1, space="PSUM") as psC,
                tc.tile_pool(name="psB3", bufs=1, space="PSUM") as psB3,
            ):
                def a_epilogue(jt):
                    p = pa[jt % 4]
                    nc.vector.reciprocal(csinv[:, jt:jt + 1],
                                         p[:, DOUT:DOUT + 1])
                    # T2' = T2 / cs -> rhs cols [258:514] (pass B3 moving)
                    nc.vector.tensor_scalar_mul(
                        rhs[jt][:, T2_C:T2_C + DOUT], p[:, 0:DOUT],
                        csinv[:, jt:jt + 1])
                    # out1 = relu(0.5*(T1 + T2)); T1 = t1p * rs
                    t1f = tmpp.tile([P, DOUT], F32, tag="tm", name=f"t1f{jt}")
                    nc.gpsimd.tensor_scalar_mul(t1f, t1p[jt], rs[:, jt:jt + 1])
                    o1t = tmpp.tile([P, DOUT], F32, tag="tm", name=f"o1t{jt}")
                    nc.vector.tensor_add(o1t, t1f, p[:, 0:DOUT])
                    o1 = stage.tile([P, DOUT], F32, tag="st", name=f"o1{jt}")
                    nc.scalar.activation(o1, o1t, RELU, scale=0.5)
                    nc.scalar.dma_start(out_r[:, jt, 0:DOUT], o1)

                def a_chain_full(jt):
                    p = pa[jt % 4]
                    for i in range(NO):
                        nc.tensor.matmul(
                            p[:, 0:DOUT + 1],
                            g_bf[i][:, jt * P:(jt + 1) * P],
                            rhs[i][:, 0:DOUT + 1],
                            start=(i == 0), stop=(i == NO - 1))

                for jt in range(4):
                    a_epilogue(jt)
                for jt in range(4, NO):
                    a_chain_full(jt)
                    a_epilogue(jt)

                # pass C: out2 = relu(G^T @ T1')
                c0 = psC.tile([P, 2, DOUT], F32)
                c1 = psC.tile([P, 2, DOUT], F32)
                c_slots = [c0[:, 0, :], c0[:, 1, :], c1[:, 0, :], c1[:, 1, :]]
                for jt in range(NO):
                    pc = c_slots[jt % 4]
                    for i in range(NO):
                        nc.tensor.matmul(
                            pc, g_bf[i][:, jt * P:(jt + 1) * P], t1p[i],
                            start=(i == 0), stop=(i == NO - 1))
                    o2 = stage.tile([P, DOUT], F32, tag="st", name=f"o2{jt}")
                    nc.scalar.activation(o2, pc, RELU)
                    nc.scalar.dma_start(out_r[:, jt, DOUT:2 * DOUT], o2)

                # pass B3: out3 = relu(G @ T2')
                b0 = psB3.tile([P, 2, DOUT], F32)
                b1t = psB3.tile([P, 2, DOUT], F32)
                b_slots = [b0[:, 0, :], b0[:, 1, :], b1t[:, 0, :], b1t[:, 1, :]]
                for r in range(NO):
                    pb = b_slots[r % 4]
                    for k in range(NO):
                        nc.tensor.matmul(
                            pb, gt[r][:, k, :], rhs[k][:, T2_C:T2_C + DOUT],
                            start=(k == 0), stop=(k == NO - 1))
                    o3 = stage.tile([P, DOUT], F32, tag="st", name=f"o3{r}")
                    nc.scalar.activation(o3, pb, RELU)
                    nc.scalar.dma_start(out_r[:, r, 2 * DOUT:W3], o3)

    nc.compile()
    return nc


_NC = None


def _get_nc():
    global _NC
    if _NC is None:
        _NC = build()
    return _NC


def run(inputs: dict, trace: bool = False):
    """Run on 8 cores; returns (stacked_out [B,N,W3], BassKernelResults)."""
    H, G, W = inputs["H"], inputs["G"], inputs["W"]
    H = np.ascontiguousarray(H, dtype=np.float32)
    G = np.ascontiguousarray(G, dtype=np.float32)
    W = np.ascontiguousarray(W, dtype=np.float32)
    in_maps = [
        {"G": np.ascontiguousarray(G[b]), "H": np.ascontiguousarray(H[b]), "W": W}
        for b in range(B)
    ]
    nc = _get_nc()
    res = run_bass_kernel_spmd(nc, in_maps, core_ids=list(range(B)), trace=trace)
    out = np.stack([res.results[b]["out"] for b in range(B)], axis=0)
    return out, res


def kernel(H, G, W):
    out, _ = run({"H": H, "G": G, "W": W})
    return out


# revision 16
# speedup vs baseline: 1.1181x; 1.0093x over previous
"""DirectedGraphConvolution Trainium2 kernel (bf16 streaming rewrite).

Per batch element b (one per NeuronCore, 8 total, data-parallel):
    N_e = H @ W                          [n, dout]
    T1  = G  @ N_e                       [n, dout]
    T2  = G.T @ N_e   (+ cs via a fused ones-column)
    out = [ relu(0.5*(T1 + T2)),             # G_F @ N_e
            relu(G.T @ (T1 / rs[:,None])),   # G_Sin @ N_e
            relu(G  @ (T2 / cs[:,None])) ]   # G_So  @ N_e

All matmuls run in bf16 (1 cyc/row on PE for matmul AND transpose; fp32
psum accumulate; scale-rel err ~3e-3).  G converts f32->bf16 on arrival
on the ACT engine, whose accum_out gives the row sums rs for free; both
G (natural) and G^T (PE-transposed) then stay SBUF-resident in bf16
(8 MB each), so the G-left products (T1, out3) never re-transpose.  cs
falls out of a ones-column fused into pass A's moving operand (col 256).

Schedule (PE p-state: any idle gap halves the clock for 3us, so the PE
queue is ordered to never block): during the ~55us G stream the PE does,
per arriving tile i: 16 G-block transposes, pass-A chain steps for
source i-4 (4 of 16 chains), the H@W pipeline lumped in blocks 4-7
(H's 1KB-descriptor DMA is slow to land, so nothing touches H before
~block 4), and two streamed T1 row chains per block from block 7.  Post
stream: the 12 remaining pass-A chains + epilogues (T2', cs, out1), then
pass C (G^T @ T1'), then pass B3 (G @ T2') back-to-back.  Identity
matrices arrive via a DMA'd EYE input (gpsimd is 18x too slow for its
usual iota job and is not used at all).  Outputs leave on the scalar
queue; all inputs use the sync queue except EYE/W/G0/G1 which sneak in
on scalar so PE work starts early.
"""

import numpy as np
import concourse.bass as bass
import concourse.mybir as mybir
import concourse.tile as tile
from concourse import bacc
from concourse.bass_utils import run_bass_kernel_spmd

F32 = mybir.dt.float32
BF16 = mybir.dt.bfloat16
RELU = mybir.ActivationFunctionType.Relu
COPY = mybir.ActivationFunctionType.Copy

P = 128
B = 8
N = 2048
NO = N // P            # 16 row tiles
HNO = N // 2           # half-tile columns
DIN = 256
DOUT = 256
KO = DIN // P          # 2 k tiles for H @ W
W3 = 3 * DOUT
RW = 2 * DOUT + 2      # rhs cols: [N_e(256) | one | pad | T2'(256)]
ONE_C = 256
T2_C = 258

A_LAG = 5              # pass-A chain source lag behind the arrival block
# streamed T1 (B1) rows per block: all of N_e exists by block 7
B1_DUE = {7: [0, 1], 8: [2, 3], 9: [4, 5], 10: [6, 7], 11: [8, 9],
          12: [10, 11], 13: [12, 13], 14: [14], 15: [15]}


def build():
    nc = bacc.Bacc("TRN2", target_bir_lowering=False)
    G = nc.declare_dram_parameter("G", [N, N], F32, isOutput=False)
    H = nc.declare_dram_parameter("H", [N, DIN], F32, isOutput=False)
    W = nc.declare_dram_parameter("W", [DIN, DOUT], F32, isOutput=False)
    EYE = nc.declare_dram_parameter("EYE", [P, P], F32, isOutput=False)
    out = nc.declare_dram_parameter("out", [N, W3], F32, isOutput=True)

    G_r = G.rearrange("(o p) j -> p o j", p=P)
    H_r = H.rearrange("(o p) d -> p o d", p=P)
    W_r = W.rearrange("(o p) d -> p o d", p=P)
    out_r = out.rearrange("(o p) d -> p o d", p=P)

    with tile.TileContext(nc) as tc:
        with (
            tc.tile_pool(name="const", bufs=1) as const,
            tc.tile_pool(name="gland", bufs=3) as gland,
            tc.tile_pool(name="gbf", bufs=1) as gbfp,
            tc.tile_pool(name="gtp", bufs=1) as gtpp,
            tc.tile_pool(name="rhs", bufs=1) as rhsp,
            tc.tile_pool(name="t1pp", bufs=1) as t1pp,
            tc.tile_pool(name="hland", bufs=1) as hlp,
            tc.tile_pool(name="htmp", bufs=4) as htp,
            tc.tile_pool(name="stage", bufs=5) as stage,
            tc.tile_pool(name="tmp", bufs=3) as tmpp,
            tc.tile_pool(name="psA", bufs=1, space="PSUM") as psA,
        ):
            # ---- constants / persistent tiles ----
            eye_f32 = const.tile([P, P], F32)
            ident = const.tile([P, P], BF16)
            w_bf = const.tile([P, KO, DOUT], BF16)
            rsh = const.tile([P, 2 * NO], F32)   # per-half row sums
            rs = const.tile([P, NO], F32)
            rsinv = const.tile([P, NO], F32)
            csinv = const.tile([P, NO], F32)

            g_bf = [gbfp.tile([P, N], BF16, tag=f"g{i}", name=f"g{i}")
                    for i in range(NO)]
            gt = [gtpp.tile([P, NO, P], BF16, tag=f"t{i}", name=f"t{i}")
                  for i in range(NO)]
            rhs = [rhsp.tile([P, RW], BF16, tag=f"r{i}", name=f"r{i}")
                   for i in range(NO)]
            t1p = [t1pp.tile([P, DOUT], BF16, tag=f"p{i}", name=f"p{i}")
                   for i in range(NO)]
            hland = [hlp.tile([P, DIN], F32, tag=f"h{t}", name=f"h{t}")
                     for t in range(NO)]
            w_land = [tmpp.tile([P, DOUT], F32, tag="tm", name=f"wl{k}")
                      for k in range(KO)]

            # ---- input DMAs ----
            # scalar queue: EYE, W, G tiles 0-1 (so PE transposes start
            # ~9us in); sync queue: all H (1KB descriptors, lands by
            # ~15us -> H work is scheduled in stream blocks 4-7), then
            # G tiles 2-15.  gland ring (3 bufs) throttles the G stream
            # to the ACT converts via slot-release deps.
            nc.scalar.dma_start(eye_f32, EYE.rearrange("a b -> a b"))
            for k in range(KO):
                nc.scalar.dma_start(w_land[k], W_r[:, k, :])
            for t in range(NO):
                nc.sync.dma_start(hland[t], H_r[:, t, :])
            g_land = []
            for i in range(NO):
                gl = gland.tile([P, N], F32, tag="gl", name=f"gl{i}")
                g_land.append(gl)
                if i < 2:
                    # halves so the first converts/transposes start early
                    nc.scalar.dma_start(gl[:, 0:HNO], G_r[:, i, 0:HNO])
                    nc.scalar.dma_start(gl[:, HNO:N], G_r[:, i, HNO:N])
                else:
                    nc.sync.dma_start(gl, G_r[:, i, :])

            nc.vector.tensor_copy(ident, eye_f32)
            for k in range(KO):
                nc.vector.tensor_copy(w_bf[:, k, :], w_land[k])
            for i in range(NO):
                nc.vector.memset(rhs[i][:, ONE_C:ONE_C + 1], 1.0)

            with (
                tc.tile_pool(name="psTR", bufs=3, space="PSUM") as psTR,
                tc.tile_pool(name="psB1", bufs=1, space="PSUM") as psB1,
            ):
                pa = [psA.tile([P, DOUT + 1], F32, tag=f"pa{j}", name=f"pa{j}")
                      for j in range(4)]
                psb1 = psB1.tile([P, 2, DOUT], F32)

                def emit_h_tr(t):
                    # transpose H tile t (f32, 2 cyc/row) into a psb1 half;
                    # DVE copies out the bf16 H^T blocks
                    half = psb1[:, t % 2, :]
                    for kt in range(KO):
                        nc.tensor.transpose(
                            half[:, kt * P:(kt + 1) * P],
                            hland[t][:, kt * P:(kt + 1) * P], eye_f32)
                    ht = htp.tile([P, KO, P], BF16, tag="ht", name=f"ht{t}")
                    nc.vector.tensor_copy(
                        ht, half.rearrange("p (k q) -> p k q", k=KO))
                    return ht

                def emit_h_ne(t, ht):
                    # N_e tile t = H^T_t.T @ W into the same psb1 half,
                    # copied to rhs (ACT; bf16)
                    pne = psb1[:, t % 2, :]
                    for kt in range(KO):
                        nc.tensor.matmul(
                            pne, ht[:, kt, :], w_bf[:, kt, :],
                            start=(kt == 0), stop=(kt == KO - 1))
                    nc.scalar.activation(rhs[t][:, 0:DOUT], pne, COPY)

                def emit_b1_row(r):
                    pb = psb1[:, r % 2, :]
                    for k in range(NO):
                        nc.tensor.matmul(
                            pb, gt[r][:, k, :], rhs[k][:, 0:DOUT],
                            start=(k == 0), stop=(k == NO - 1))
                    # T1' = T1 / rs  (bf16, pass C moving operand)
                    nc.vector.tensor_scalar_mul(t1p[r], pb, rsinv[:, r:r + 1])

                def emit_a_chain_step(ia):
                    for jt in range(4):
                        nc.tensor.matmul(
                            pa[jt][:, 0:DOUT + 1],
                            g_bf[ia][:, jt * P:(jt + 1) * P],
                            rhs[ia][:, 0:DOUT + 1],
                            start=(ia == 0), stop=(ia == NO - 1))

                # ---- stream loop (one block per arriving G tile) ----
                ht_prev = None
                tb_tiles = []
                for i in range(NO):
                    # f32 -> bf16 convert in halves (earlier transpose
                    # start); accum_out gives per-half row sums
                    nc.scalar.activation(
                        g_bf[i][:, 0:HNO], g_land[i][:, 0:HNO], COPY,
                        accum_out=rsh[:, 2 * i:2 * i + 1])
                    nc.scalar.activation(
                        g_bf[i][:, HNO:N], g_land[i][:, HNO:N], COPY,
                        accum_out=rsh[:, 2 * i + 1:2 * i + 2])
                    nc.vector.tensor_add(rs[:, i:i + 1], rsh[:, 2 * i:2 * i + 1],
                                         rsh[:, 2 * i + 1:2 * i + 2])
                    nc.vector.reciprocal(rsinv[:, i:i + 1], rs[:, i:i + 1])
                    # ACT finishes the PREVIOUS tile's gt (its psum source
                    # is long done -> no ACT head-of-line stall)
                    if 1 <= i <= 12:
                        tbp = tb_tiles[i - 1]
                        nc.scalar.activation(gt[i - 1][:, 12:16, :],
                                             tbp[:, 4:8, :], COPY)

                    # 16 PE transposes -> G^T blocks (bf16 psum), copied
                    # out in 4-block chunks: 3 on DVE, the last on ACT
                    # (next block) except near the end where B1 needs the
                    # full gt tile in-block.
                    ta = psTR.tile([P, 8, P], BF16, tag="tr", name=f"ta{i}")
                    for k in range(8):
                        nc.tensor.transpose(
                            ta[:, k, :], g_bf[i][:, k * P:(k + 1) * P], ident)
                        if k == 3:
                            nc.vector.tensor_copy(gt[i][:, 0:4, :], ta[:, 0:4, :])
                    nc.vector.tensor_copy(gt[i][:, 4:8, :], ta[:, 4:8, :])
                    if i >= A_LAG:
                        emit_a_chain_step(i - A_LAG)
                    tb = psTR.tile([P, 8, P], BF16, tag="tr", name=f"tb{i}")
                    tb_tiles.append(tb)
                    for k in range(8):
                        nc.tensor.transpose(
                            tb[:, k, :], g_bf[i][:, (8 + k) * P:(9 + k) * P],
                            ident)
                        if k == 3:
                            nc.vector.tensor_copy(gt[i][:, 8:12, :], tb[:, 0:4, :])
                    if i >= 12:
                        nc.vector.tensor_copy(gt[i][:, 12:16, :], tb[:, 4:8, :])

                    # H @ W lump: H has landed by block 4; one quad per
                    # block 4-7, software-pipelined tr/ne
                    if 4 <= i < 8:
                        for j in range(4):
                            t = 4 * (i - 4) + j
                            if ht_prev is not None:
                                emit_h_ne(t - 1, ht_prev)
                            ht_prev = emit_h_tr(t)
                        if i == 7:
                            emit_h_ne(NO - 1, ht_prev)

                    for r in B1_DUE.get(i, []):
                        emit_b1_row(r)

                # A chain catch-up (sources 12..15)
                for ia in range(NO - A_LAG, NO):
                    emit_a_chain_step(ia)

            # ---- post phase: A jt4-15 + epilogues, then C, then B3 ----
            with (
                tc.tile_pool(name="psC", bufs=1, space="PSUM") as psC,
                tc.tile_pool(name="psB3", bufs=1, space="PSUM") as psB3,
            ):
                def a_epilogue(jt):
                    p = pa[jt % 4]
                    nc.vector.reciprocal(csinv[:, jt:jt + 1],
                                         p[:, DOUT:DOUT + 1])
                    # T2' = T2 / cs -> rhs cols [258:514] (pass B3 moving)
                    nc.vector.tensor_scalar_mul(
                        rhs[jt][:, T2_C:T2_C + DOUT], p[:, 0:DOUT],
                        csinv[:, jt:jt + 1])
                    # out1 = relu(0.5*(T1 + T2)); T1 = t1p * rs
                    t1f = tmpp.tile([P, DOUT], F32, tag="tm", name=f"t1f{jt}")
                    nc.vector.tensor_scalar_mul(t1f, t1p[jt], rs[:, jt:jt + 1])
                    o1t = tmpp.tile([P, DOUT], F32, tag="tm", name=f"o1t{jt}")
                    nc.vector.tensor_add(o1t, t1f, p[:, 0:DOUT])
                    o1 = stage.tile([P, DOUT], F32, tag="st", name=f"o1{jt}")
                    nc.scalar.activation(o1, o1t, RELU, scale=0.5)
                    nc.scalar.dma_start(out_r[:, jt, 0:DOUT], o1)

                def a_chain_full(jt):
                    p = pa[jt % 4]
                    for i in range(NO):
                        nc.tensor.matmul(
                            p[:, 0:DOUT + 1],
                            g_bf[i][:, jt * P:(jt + 1) * P],
                            rhs[i][:, 0:DOUT + 1],
                            start=(i == 0), stop=(i == NO - 1))

                for jt in range(4):
                    a_epilogue(jt)
                for jt in range(4, NO):
                    a_chain_full(jt)
                    a_epilogue(jt)

                # pass C: out2 = relu(G^T @ T1')
                c0 = psC.tile([P, 2, DOUT], F32)
                c1 = psC.tile([P, 2, DOUT], F32)
                c_slots = [c0[:, 0, :], c0[:, 1, :], c1[:, 0, :], c1[:, 1, :]]
                for jt in range(NO):
                    pc = c_slots[jt % 4]
                    for i in range(NO):
                        nc.tensor.matmul(
                            pc, g_bf[i][:, jt * P:(jt + 1) * P], t1p[i],
                            start=(i == 0), stop=(i == NO - 1))
                    o2 = stage.tile([P, DOUT], F32, tag="st", name=f"o2{jt}")
                    nc.scalar.activation(o2, pc, RELU)
                    nc.scalar.dma_start(out_r[:, jt, DOUT:2 * DOUT], o2)

                # pass B3: out3 = relu(G @ T2')
                b0 = psB3.tile([P, 2, DOUT], F32)
                b1t = psB3.tile([P, 2, DOUT], F32)
                b_slots = [b0[:, 0, :], b0[:, 1, :], b1t[:, 0, :], b1t[:, 1, :]]
                for r in range(NO):
                    pb = b_slots[r % 4]
                    for k in range(NO):
                        nc.tensor.matmul(
                            pb, gt[r][:, k, :], rhs[k][:, T2_C:T2_C + DOUT],
                            start=(k == 0), stop=(k == NO - 1))
                    o3 = stage.tile([P, DOUT], F32, tag="st", name=f"o3{r}")
                    nc.scalar.activation(o3, pb, RELU)
                    nc.scalar.dma_start(out_r[:, r, 2 * DOUT:W3], o3)

    nc.compile()
    return nc


_NC = None
_EYE = None


def _get_nc():
    global _NC, _EYE
    if _NC is None:
        _NC = build()
        _EYE = np.eye(P, dtype=np.float32)
    return _NC


def run(inputs: dict, trace: bool = False):
    """Run on 8 cores; returns (stacked_out [B,N,W3], BassKernelResults)."""
    H, G, W = inputs["H"], inputs["G"], inputs["W"]
    H = np.ascontiguousarray(H, dtype=np.float32)
    G = np.ascontiguousarray(G, dtype=np.float32)
    W = np.ascontiguousarray(W, dtype=np.float32)
    nc = _get_nc()
    in_maps = [
        {"G": np.ascontiguousarray(G[b]), "H": np.ascontiguousarray(H[b]),
         "W": W, "EYE": _EYE}
        for b in range(B)
    ]
    res = run_bass_kernel_spmd(nc, in_maps, core_ids=list(range(B)), trace=trace)
    out = np.stack([res.results[b]["out"] for b in range(B)], axis=0)
    return out, res


def kernel(H, G, W):
    out, _ = run({"H": H, "G": G, "W": W})
    return out
